# revision 5
# baseline (speedup 1.0000x reference)
"""Single-head attention (B=8, T=2048, E=1024, D=128) on 8 Trainium2 NeuronCores.

Strategy (data-parallel over batch, one batch element per core):
  host: pre-transpose x -> xT[b] = x[b].T (E on rows); pre-scale handled by
        folding D**-0.25 into the q/k bias-add copies.
  device, per core (all matmul operands fp16; PSUM f32):
    - qT/kT/vT = W.T @ xT via PE (fp16 matmuls, N=512 quarters, accumulate
      over 8 E-chunks in PSUM); biases folded into the PSUM->SBUF copy.
    - V (natural [k, D] layout) from vT via 16 fp16 PE transposes.
    - per 512-wide query span:
        for each pair of key blocks: two score matmuls into a 2-bank PSUM
        pair tile; ONE exp on ACT [128, 1024] -> P16[:, 2p:2p+2, :] fp16
        PV: OT[d, q] += V_blk.T @ P_blk (16 matmuls, accumulated in PSUM)
        denominators: fp16 fold-tree on DVE (16 blocks -> 1) then a single
        ones[128x128] matmul per span => l[*, q] broadcast across partitions
        out_span = OT * approx(1/l) on DVE
    - store outT [D, T]; host transposes back to [T, D].
"""

import os
import sys

for _p in ("/opt/trn_rl_repo",):
    if _p not in sys.path and os.path.isdir(_p):
        sys.path.append(_p)

import numpy as np

import concourse.bass as bass
import concourse.tile as tile
from concourse import mybir
from concourse.vector_clock import ScopedClock

B, T, E, D = 8, 2048, 1024, 128
EC = E // 128          # E chunks of 128 partitions
NSPAN = 4              # query spans of 512
SPAN = T // NSPAN      # 512
NKB = T // 128         # 16 key blocks
NPAIR = NKB // 2       # 8 key-block pairs
F32 = mybir.dt.float32
F32R = mybir.dt.float32r
BF16 = mybir.dt.bfloat16
F16 = mybir.dt.float16

_MAX_DRAIN_WAITS = 1


def _drain_and_barrier_split(self, tick_clock, wait_clock):
    # This walrus build rejects CTRL instructions carrying more than one sync
    # wait, so spread the kernel-tail drain's waits over single-wait NOPs.
    nc = self.nc
    collector = nc.sync.nop(nofuse=True, hint="drain_wait_collector")
    wait_clock.add_sem_waits(
        collector.ins, ScopedClock({None: tick_clock.global_clock})
    )
    si = collector.ins.sync_info
    waits = list(si.on_wait) if si and si.on_wait else []
    if len(waits) > _MAX_DRAIN_WAITS:
        si.on_wait = waits[:_MAX_DRAIN_WAITS]
        rest = waits[_MAX_DRAIN_WAITS:]
        while rest:
            chunk, rest = rest[:_MAX_DRAIN_WAITS], rest[_MAX_DRAIN_WAITS:]
            extra = nc.sync.nop(nofuse=True, hint="drain_wait_extra")
            if extra.ins.sync_info is None:
                extra.ins.sync_info = type(si)(on_wait=chunk, on_update=[])
            else:
                extra.ins.sync_info.on_wait = chunk

    nc.sync.drain()

    nc.all_engine_barrier()
    assert self.sems is not None
    popped = nc._tile_sem_poison_stack.pop()
    assert popped is self._sem_poison
    nc.clear_and_free_semaphores(list(self.sems.allocated().values()))
    nc.all_engine_barrier()


tile.TileContext._drain_and_barrier = _drain_and_barrier_split


def _split_excess_waits(nc):
    """Walrus in this env allows at most one sync wait per instruction;
    hoist extra waits onto same-engine NOPs placed just before."""
    import copy

    m = nc.m
    cnt = 0
    new_funcs = []
    for function in m.functions:
        new_function = copy.replace(function, blocks=[])
        new_function.set_allocations_from_list(function.allocations)
        for block in function.blocks:
            new_insts = []
            for inst in block.instructions:
                si = inst.sync_info
                waits = list(si.on_wait) if si and si.on_wait else []
                if len(waits) > 1:
                    for w in waits[:-1]:
                        nop = mybir.InstNoOp(name=f"I-swsplit-{cnt}",
                                             ins=[], outs=[])
                        cnt += 1
                        nop.engine = inst.engine
                        nop.sync_info = mybir.SyncInfo(on_wait=[w],
                                                       on_update=[])
                        new_insts.append(nop)
                    si.on_wait = [waits[-1]]
                new_insts.append(inst)
            new_function.blocks.append(
                copy.replace(block, instructions=new_insts))
        new_funcs.append(new_function)
    new_m = copy.replace(m, functions=[])
    for f in new_funcs:
        new_m.functions.append(f)
    nc.m = new_m
    return cnt


def build_nc():
    SCALE = float(np.float32(D) ** np.float32(-0.25))

    nc = bass.Bass()
    xT = nc.declare_dram_parameter("xT", [E, T], F16, isOutput=False)[:]
    Wq = nc.declare_dram_parameter("Wq", [128, EC * D], F16, isOutput=False)[:]
    Wk = nc.declare_dram_parameter("Wk", [128, EC * D], F16, isOutput=False)[:]
    Wv = nc.declare_dram_parameter("Wv", [128, EC * D], F16, isOutput=False)[:]
    bqc = nc.declare_dram_parameter("bqc", [D], F32, isOutput=False)[:]
    bkc = nc.declare_dram_parameter("bkc", [D], F32, isOutput=False)[:]
    bv = nc.declare_dram_parameter("bv", [D], F32, isOutput=False)[:]
    ident_d = nc.declare_dram_parameter("ident", [128, 128], F16,
                                        isOutput=False)[:]
    ones_d = nc.declare_dram_parameter("ones", [128, 128], F16,
                                       isOutput=False)[:]
    outT = nc.declare_dram_parameter("outT", [D, T], F32, isOutput=True)[:]

    with tile.TileContext(nc) as tc, \
         tc.tile_pool(name="consts", bufs=1) as consts, \
         tc.tile_pool(name="xpool", bufs=1) as xpool, \
         tc.tile_pool(name="persist", bufs=1) as persist, \
         tc.tile_pool(name="ppool", bufs=2) as ppool, \
         tc.tile_pool(name="tpool", bufs=2) as tpool, \
         tc.tile_pool(name="lbpool", bufs=2) as lbpool, \
         tc.tile_pool(name="outpool", bufs=2) as outpool, \
         tc.tile_pool(name="psProj", bufs=2, space="PSUM") as psProj, \
         tc.tile_pool(name="psPair", bufs=2, space="PSUM") as psPair, \
         tc.tile_pool(name="psTp", bufs=1, space="PSUM") as psTp, \
         tc.tile_pool(name="psOT", bufs=1, space="PSUM") as psOT:

        # ---- constants / weights ----
        wq_s = consts.tile([128, EC, D], F16, tag="wq")
        wk_s = consts.tile([128, EC, D], F16, tag="wk")
        wv_s = consts.tile([128, EC, D], F16, tag="wv")
        wk_r = Wk.rearrange("p (c d) -> p c d", d=D)
        nc.sync.dma_start(out=wk_s[:, 0:2, :], in_=wk_r[:, 0:2, :])
        bq_s = consts.tile([128, 1], F32, tag="bq")
        bk_s = consts.tile([128, 1], F32, tag="bk")
        bv_s = consts.tile([128, 1], F32, tag="bv")
        ident = consts.tile([128, 128], F16, tag="ident")
        ones_mat = consts.tile([128, 128], F16, tag="ones_mat")

        # ---- x pieces (E-partitioned, quarter-T granularity), streamed in
        # consumption order and interleaved with the remaining weights ----
        Q4 = T // 4
        xp = [[None] * 4 for _ in range(EC)]
        for h in range(4):
            for e in range(EC):
                t = xpool.tile([128, Q4], F16, tag=f"xp{e}_{h}")
                nc.sync.dma_start(
                    out=t,
                    in_=xT[e * 128:(e + 1) * 128, h * Q4:(h + 1) * Q4])
                xp[e][h] = t
                if h == 0 and e == 0:
                    nc.sync.dma_start(out=wk_s[:, 2:EC, :],
                                      in_=wk_r[:, 2:EC, :])
                    nc.sync.dma_start(
                        out=wv_s, in_=Wv.rearrange("p (c d) -> p c d", d=D))
                    nc.sync.dma_start(
                        out=wq_s, in_=Wq.rearrange("p (c d) -> p c d", d=D))
                if h == 0 and e == 1:
                    # small constants, needed only after the first quarter
                    for b_s, b_d in ((bq_s, bqc), (bk_s, bkc), (bv_s, bv)):
                        nc.sync.dma_start(out=b_s, in_=b_d.unsqueeze(1))
                    nc.sync.dma_start(out=ident, in_=ident_d)
                    nc.sync.dma_start(out=ones_mat, in_=ones_d)

        kT_s = persist.tile([128, T], F16, tag="kT")
        vT_s = persist.tile([128, T], F16, tag="vT")
        qT_s = persist.tile([128, T], F16, tag="qT")
        V_s = persist.tile([128, NKB, D], F16, tag="V")

        # ---- q/k/v projections, T-quarters so PE consumes chunks as they
        # arrive (accumulate over E in PSUM) ----
        for h in range(4):
            k_ps = psProj.tile([128, T // 4], F32, tag="mm")
            v_ps = psProj.tile([128, T // 4], F32, tag="mm")
            q_ps = psProj.tile([128, T // 4], F32, tag="mm")
            hsl = slice(h * (T // 4), (h + 1) * (T // 4))
            for e in range(EC):
                xsl = xp[e][h]
                for w_s, ps in ((wk_s, k_ps), (wv_s, v_ps), (wq_s, q_ps)):
                    nc.tensor.matmul(ps, w_s[:, e, :], xsl,
                                     start=(e == 0), stop=(e == EC - 1))
            nc.vector.tensor_scalar(out=kT_s[:, hsl], in0=k_ps,
                                    scalar1=bk_s, scalar2=SCALE,
                                    op0=mybir.AluOpType.add,
                                    op1=mybir.AluOpType.mult)
            nc.vector.tensor_scalar(out=vT_s[:, hsl], in0=v_ps,
                                    scalar1=bv_s, scalar2=None,
                                    op0=mybir.AluOpType.add)
            nc.vector.tensor_scalar(out=qT_s[:, hsl], in0=q_ps,
                                    scalar1=bq_s, scalar2=SCALE,
                                    op0=mybir.AluOpType.add,
                                    op1=mybir.AluOpType.mult)

        # ---- V natural layout [k, D] via fp16 PE transposes of vT ----
        for t in range(0, NKB, 2):
            vt_ps = psTp.tile([128, 2, 128], F16, tag="tp")
            for i in range(2):
                nc.tensor.transpose(vt_ps[:, i, :],
                                    vT_s[:, (t + i) * 128:(t + i + 1) * 128],
                                    ident)
            nc.vector.tensor_copy(out=V_s[:, t:t + 2, :], in_=vt_ps)

        # ---- per query span: scores^T (pairs), exp, PV, denom tree ----
        for s in range(NSPAN):
            ssl = slice(s * SPAN, (s + 1) * SPAN)
            ot_ps = psOT.tile([128, SPAN], F32, tag="ot")
            P16 = ppool.tile([128, NKB, SPAN], F16, tag="p16")
            prev = None
            for p in range(NPAIR):
                st_ps = psPair.tile([128, 2, SPAN], F32, tag="pair")
                for i in range(2):
                    kb = 2 * p + i
                    nc.tensor.matmul(st_ps[:, i, :],
                                     kT_s[:, kb * 128:(kb + 1) * 128],
                                     qT_s[:, ssl], start=True, stop=True)
                nc.scalar.activation(out=P16[:, 2 * p:2 * p + 2, :],
                                     in_=st_ps,
                                     func=mybir.ActivationFunctionType.Exp)
                if prev is not None:
                    for i in range(2):
                        kb = 2 * prev + i
                        nc.tensor.matmul(ot_ps, V_s[:, kb, :], P16[:, kb, :],
                                         start=(kb == 0), stop=False)
                prev = p
            for i in range(2):
                kb = 2 * prev + i
                nc.tensor.matmul(ot_ps, V_s[:, kb, :], P16[:, kb, :],
                                 start=False, stop=(kb == NKB - 1))

            # denominator: fold-tree on DVE (fp16, 2x mode) then one matmul
            tree = tpool.tile([128, NKB // 2, SPAN], F16, tag="tree")
            nc.vector.tensor_add(out=tree, in0=P16[:, 0:8, :],
                                 in1=P16[:, 8:16, :])
            nc.vector.tensor_add(out=tree[:, 0:4, :], in0=tree[:, 0:4, :],
                                 in1=tree[:, 4:8, :])
            nc.vector.tensor_add(out=tree[:, 0:2, :], in0=tree[:, 0:2, :],
                                 in1=tree[:, 2:4, :])
            nc.vector.tensor_add(out=tree[:, 0, :], in0=tree[:, 0, :],
                                 in1=tree[:, 1, :])
            lb_ps = psProj.tile([128, SPAN], F32, tag="mm")
            nc.tensor.matmul(lb_ps, ones_mat, tree[:, 0, :],
                             start=True, stop=True)

            lb = lbpool.tile([128, SPAN], F32, tag="lb")
            nc.vector.reciprocal(out=lb, in_=lb_ps)
            outsp = outpool.tile([128, SPAN], F32, tag="out")
            nc.vector.tensor_mul(out=outsp, in0=ot_ps, in1=lb)
            nc.sync.dma_start(out=outT[:, ssl], in_=outsp)

    return nc


_CACHED = {}


def _get_nc(key="fp16"):
    if key not in _CACHED:
        nc = build_nc()
        _split_excess_waits(nc)
        _CACHED[key] = nc
    return _CACHED[key]


def _make_in_maps(x, Wq, bq, Wk, bk, Wv, bv):
    def rnd(a):
        return np.ascontiguousarray(np.asarray(a, np.float32), np.float16)

    xT = rnd(np.transpose(np.asarray(x, np.float32), (0, 2, 1)))

    def warr(w):
        w = np.asarray(w, np.float32).reshape(EC, 128, D)
        return rnd(w.transpose(1, 0, 2).reshape(128, EC * D))

    Wq, Wk, Wv = warr(Wq), warr(Wk), warr(Wv)
    bqc = np.ascontiguousarray(np.asarray(bq, np.float32))
    bkc = np.ascontiguousarray(np.asarray(bk, np.float32))
    bv = np.ascontiguousarray(np.asarray(bv, np.float32))
    ident = np.eye(128, dtype=np.float16)
    ones = np.ones((128, 128), np.float16)
    return [
        {"xT": np.ascontiguousarray(xT[b]), "Wq": Wq, "Wk": Wk, "Wv": Wv,
         "bqc": bqc, "bkc": bkc, "bv": bv, "ident": ident, "ones": ones}
        for b in range(B)
    ]


def kernel(x, Wq, bq, Wk, bk, Wv, bv, _trace=False, _mm_dt=None):
    from concourse.bass_utils import run_bass_kernel_spmd

    nc = _get_nc()
    in_maps = _make_in_maps(x, Wq, bq, Wk, bk, Wv, bv)
    res = run_bass_kernel_spmd(nc, in_maps, core_ids=list(range(B)),
                               trace=_trace)
    out = np.stack([np.ascontiguousarray(res.results[b]["outT"].T)
                    for b in range(B)])
    kernel._last_result = res
    return out


# revision 16
# speedup vs baseline: 1.0258x; 1.0258x over previous
"""Single-head attention (B=8, T=2048, E=1024, D=128) on 8 Trainium2 NeuronCores.

Strategy (data-parallel over batch, one batch element per core, all-fp16 PE):
  host: pre-transpose x -> xT[b] = x[b].T (E on rows).
  device, per core, software-pipelined so the ACT (exp) stream starts early
  and PE never idles:
    - quarter h: kT/vT projections (fp16 matmuls over 8 E-chunks, PSUM);
      quarter 0 also projects qT[0:512]; span-0 score pairs chase the kT
      quarters (flash-style over k).
    - V (natural [k, D]) from vT via DMA XBAR transposes (no PE/DVE cost).
    - per 512-wide query span: per key-block pair, two score matmuls into a
      2-bank PSUM tile; one exp [128, 1024] on ACT -> P16 fp16; PV matmuls
      (lagged one pair) accumulate OT in PSUM; DVE folds P16 into per-span
      column sums (fp16 2x); a single ones[128,128] matmul per span gives the
      softmax denominators broadcast across partitions; reciprocal + multiply
      on DVE normalize. q-quarters 1-3 projections + denominator matmuls are
      interleaved into the span slots as PE filler.
    - store outT [D, T]; host transposes back to [T, D].
"""

import os
import sys

for _p in ("/opt/trn_rl_repo",):
    if _p not in sys.path and os.path.isdir(_p):
        sys.path.append(_p)

import numpy as np

import concourse.bass as bass
import concourse.tile as tile
from concourse import mybir
from concourse.vector_clock import ScopedClock

B, T, E, D = 8, 2048, 1024, 128
EC = E // 128          # E chunks of 128 partitions
NSPAN = 4              # query spans of 512
SPAN = T // NSPAN      # 512
NKB = T // 128         # 16 key blocks
NPAIR = NKB // 2       # 8 key-block pairs
F32 = mybir.dt.float32
F32R = mybir.dt.float32r
BF16 = mybir.dt.bfloat16
F16 = mybir.dt.float16

_MAX_DRAIN_WAITS = 1


def _drain_and_barrier_split(self, tick_clock, wait_clock):
    # This walrus build rejects CTRL instructions carrying more than one sync
    # wait, so spread the kernel-tail drain's waits over single-wait NOPs.
    nc = self.nc
    collector = nc.sync.nop(nofuse=True, hint="drain_wait_collector")
    wait_clock.add_sem_waits(
        collector.ins, ScopedClock({None: tick_clock.global_clock})
    )
    si = collector.ins.sync_info
    waits = list(si.on_wait) if si and si.on_wait else []
    if len(waits) > _MAX_DRAIN_WAITS:
        si.on_wait = waits[:_MAX_DRAIN_WAITS]
        rest = waits[_MAX_DRAIN_WAITS:]
        while rest:
            chunk, rest = rest[:_MAX_DRAIN_WAITS], rest[_MAX_DRAIN_WAITS:]
            extra = nc.sync.nop(nofuse=True, hint="drain_wait_extra")
            if extra.ins.sync_info is None:
                extra.ins.sync_info = type(si)(on_wait=chunk, on_update=[])
            else:
                extra.ins.sync_info.on_wait = chunk

    nc.sync.drain()

    nc.all_engine_barrier()
    assert self.sems is not None
    popped = nc._tile_sem_poison_stack.pop()
    assert popped is self._sem_poison
    nc.clear_and_free_semaphores(list(self.sems.allocated().values()))
    nc.all_engine_barrier()


tile.TileContext._drain_and_barrier = _drain_and_barrier_split


def _split_excess_waits(nc):
    """Walrus in this env allows at most one sync wait per instruction;
    hoist extra waits onto same-engine NOPs placed just before."""
    import copy

    m = nc.m
    cnt = 0
    new_funcs = []
    for function in m.functions:
        new_function = copy.replace(function, blocks=[])
        new_function.set_allocations_from_list(function.allocations)
        for block in function.blocks:
            new_insts = []
            for inst in block.instructions:
                si = inst.sync_info
                waits = list(si.on_wait) if si and si.on_wait else []
                if len(waits) > 1:
                    for w in waits[:-1]:
                        nop = mybir.InstNoOp(name=f"I-swsplit-{cnt}",
                                             ins=[], outs=[])
                        cnt += 1
                        nop.engine = inst.engine
                        nop.sync_info = mybir.SyncInfo(on_wait=[w],
                                                       on_update=[])
                        new_insts.append(nop)
                    si.on_wait = [waits[-1]]
                new_insts.append(inst)
            new_function.blocks.append(
                copy.replace(block, instructions=new_insts))
        new_funcs.append(new_function)
    new_m = copy.replace(m, functions=[])
    for f in new_funcs:
        new_m.functions.append(f)
    nc.m = new_m
    return cnt


def build_nc():
    SCALE = float(np.float32(D) ** np.float32(-0.25))

    nc = bass.Bass()
    xT = nc.declare_dram_parameter("xT", [E, T], F16, isOutput=False)[:]
    Wq = nc.declare_dram_parameter("Wq", [128, EC * D], F16, isOutput=False)[:]
    Wk = nc.declare_dram_parameter("Wk", [128, EC * D], F16, isOutput=False)[:]
    Wv = nc.declare_dram_parameter("Wv", [128, EC * D], F16, isOutput=False)[:]
    bqc = nc.declare_dram_parameter("bqc", [D], F32, isOutput=False)[:]
    bkc = nc.declare_dram_parameter("bkc", [D], F32, isOutput=False)[:]
    bv = nc.declare_dram_parameter("bv", [D], F32, isOutput=False)[:]
    ones_d = nc.declare_dram_parameter("ones", [128, 128], F16,
                                       isOutput=False)[:]
    outT = nc.declare_dram_parameter("outT", [D, T], F32, isOutput=True)[:]

    xT_r = xT.rearrange("(c p) t -> p c t", p=128)

    with tile.TileContext(nc) as tc, \
         tc.tile_pool(name="consts", bufs=1) as consts, \
         tc.tile_pool(name="xpool", bufs=1) as xpool, \
         tc.tile_pool(name="persist", bufs=1) as persist, \
         tc.tile_pool(name="ppool", bufs=2) as ppool, \
         tc.tile_pool(name="s2pool", bufs=2) as s2pool, \
         tc.tile_pool(name="lbpool", bufs=2) as lbpool, \
         tc.tile_pool(name="outpool", bufs=2) as outpool, \
         tc.tile_pool(name="psProj", bufs=2, space="PSUM") as psProj, \
         tc.tile_pool(name="psPair", bufs=2, space="PSUM") as psPair, \
         tc.tile_pool(name="psOT", bufs=2, space="PSUM") as psOT:

        # ---- weights / constants ----
        wq_s = consts.tile([128, EC, D], F16, tag="wq")
        wk_s = consts.tile([128, EC, D], F16, tag="wk")
        wv_s = consts.tile([128, EC, D], F16, tag="wv")
        bq_s = consts.tile([128, 1], F32, tag="bq")
        bk_s = consts.tile([128, 1], F32, tag="bk")
        bv_s = consts.tile([128, 1], F32, tag="bv")
        ones_mat = consts.tile([128, 128], F16, tag="ones_mat")

        Q4 = T // 4
        xq = [xpool.tile([128, EC, Q4], F16, tag=f"xq{h}", name=f"xq{h}")
              for h in range(4)]

        # DMA issue order on SP: wk, x q0, wq, wv, x q1-3, small consts.
        nc.sync.dma_start(out=wk_s, in_=Wk.rearrange("p (c d) -> p c d", d=D))
        nc.sync.dma_start(out=xq[0], in_=xT_r[:, :, 0:Q4])
        nc.sync.dma_start(out=wq_s, in_=Wq.rearrange("p (c d) -> p c d", d=D))
        nc.sync.dma_start(out=wv_s, in_=Wv.rearrange("p (c d) -> p c d", d=D))
        for h in range(1, 4):
            nc.sync.dma_start(out=xq[h], in_=xT_r[:, :, h * Q4:(h + 1) * Q4])
        for b_s, b_d in ((bk_s, bkc), (bq_s, bqc), (bv_s, bv)):
            nc.sync.dma_start(out=b_s, in_=b_d.unsqueeze(1))
        nc.sync.dma_start(out=ones_mat, in_=ones_d)

        kT_s = persist.tile([128, T], F16, tag="kT")
        vT_s = persist.tile([128, T], F16, tag="vT")
        qT_s = persist.tile([128, T], F16, tag="qT")
        V_s = persist.tile([128, NKB, D], F16, tag="V")

        def proj_quarter(w_s, h, dst, bias, scale):
            ps = psProj.tile([128, Q4], F32, tag="mm", name="proj_ps")
            hsl = slice(h * Q4, (h + 1) * Q4)
            for e in range(EC):
                nc.tensor.matmul(ps, w_s[:, e, :], xq[h][:, e, :],
                                 start=(e == 0), stop=(e == EC - 1))
            if scale is not None:
                nc.vector.tensor_scalar(out=dst[:, hsl], in0=ps, scalar1=bias,
                                  scalar2=scale, op0=mybir.AluOpType.add,
                                  op1=mybir.AluOpType.mult)
            else:
                nc.vector.tensor_scalar(out=dst[:, hsl], in0=ps, scalar1=bias,
                                        scalar2=None,
                                        op0=mybir.AluOpType.add)
            if dst is vT_s:
                # V natural layout via DMA XBAR transposes (SBUF->SBUF fp16,
                # issued on SP so the ACT sequencer never blocks on them)
                for kb in range(4 * h, 4 * h + 4):
                    nc.sync.dma_start(
                        out=V_s[:, kb, :],
                        in_=vT_s[:, kb * 128:(kb + 1) * 128],
                        transpose=True)

        # ---- PE filler machinery: deferred single-matmul emitters ----
        filler = []

        def add_q_quarter(h):
            ps_box = {}

            def mk(e):
                def emit():
                    if e == 0:
                        ps_box["ps"] = psProj.tile([128, Q4], F32, tag="mm", name="qq_ps")
                    nc.tensor.matmul(ps_box["ps"], wq_s[:, e, :],
                                     xq[h][:, e, :], start=(e == 0),
                                     stop=(e == EC - 1))
                    if e == EC - 1:
                        proj_done(h, ps_box["ps"])
                return emit

            def proj_done(h, ps):
                hsl = slice(h * Q4, (h + 1) * Q4)
                nc.vector.tensor_scalar(out=qT_s[:, hsl], in0=ps,
                                        scalar1=bq_s, scalar2=SCALE,
                                        op0=mybir.AluOpType.add,
                                        op1=mybir.AluOpType.mult)
            for e in range(EC):
                filler.append(mk(e))

        def drain_filler(n):
            for _ in range(min(n, len(filler))):
                filler.pop(0)()

        # ---- span pipeline ----
        span_state = {}

        def span_scores(s, p):
            """Emit the two score matmuls + paired exp + DVE pair-sum for
            span s, key-block pair p."""
            st = span_state[s]
            ssl = st["ssl"]
            st_ps = psPair.tile([128, 2, SPAN], F32, tag="pair", name="st_ps")
            for i in range(2):
                kb = 2 * p + i
                nc.tensor.matmul(st_ps[:, i, :],
                                 kT_s[:, kb * 128:(kb + 1) * 128],
                                 qT_s[:, ssl], start=True, stop=True)
            nc.scalar.activation(out=st["P16"][:, 2 * p:2 * p + 2, :],
                                 in_=st_ps,
                                 func=mybir.ActivationFunctionType.Exp)
            nc.vector.tensor_add(out=st["sum2"][:, p, :],
                                 in0=st["P16"][:, 2 * p, :],
                                 in1=st["P16"][:, 2 * p + 1, :])

        def span_pv(s, p=None):
            st = span_state[s]
            if p is None:
                p = st["pv_next"]
            assert p == st["pv_next"], f"PV order violation s={s} p={p}"
            st["pv_next"] += 1
            for i in range(2):
                kb = 2 * p + i
                nc.tensor.matmul(st["ot"], V_s[:, kb, :],
                                 st["P16"][:, kb, :],
                                 start=(kb == 0), stop=(kb == NKB - 1))

        def span_open(s):
            span_state[s] = {
                "ssl": slice(s * SPAN, (s + 1) * SPAN),
                "ot": psOT.tile([128, SPAN], F32, tag="ot", name="ot_ps"),
                "P16": ppool.tile([128, NKB, SPAN], F16, tag="p16", name="P16"),
                "sum2": s2pool.tile([128, NPAIR, SPAN], F16, tag="s2", name="sum2"),
                "pv_next": 0,
            }

        def span_close(s, defer_lb=True):
            """Fold pair-sums, then (deferred) denominator matmul, reciprocal,
            normalize, output DMA."""
            st = span_state[s]
            sum2 = st["sum2"]
            nc.vector.tensor_add(out=sum2[:, 0:4, :], in0=sum2[:, 0:4, :],
                                 in1=sum2[:, 4:8, :])
            nc.vector.tensor_add(out=sum2[:, 0:2, :], in0=sum2[:, 0:2, :],
                                 in1=sum2[:, 2:4, :])
            nc.vector.tensor_add(out=sum2[:, 0, :], in0=sum2[:, 0, :],
                                 in1=sum2[:, 1, :])

            def emit_lb():
                lb_ps = psProj.tile([128, SPAN], F32, tag="mm", name="lb_ps")
                nc.tensor.matmul(lb_ps, ones_mat, sum2[:, 0, :],
                                 start=True, stop=True)
                lb = lbpool.tile([128, SPAN], F32, tag="lb", name="lb")
                nc.vector.reciprocal(out=lb, in_=lb_ps)
                outsp = outpool.tile([128, SPAN], F32, tag="out", name="outsp")
                nc.vector.tensor_mul(out=outsp, in0=st["ot"], in1=lb)
                nc.sync.dma_start(out=outT[:, st["ssl"]], in_=outsp)
            if defer_lb:
                # priority: must be emitted promptly -- later spans' PV
                # accumulators WAR-wait on this span's normalize
                filler.insert(0, emit_lb)
            else:
                emit_lb()

        # ================= emission schedule =================
        # Pre-phase + flash span 0: score pairs chase the kT quarters.
        span_open(0)
        proj_quarter(wk_s, 0, kT_s, bk_s, SCALE)
        proj_quarter(wv_s, 0, vT_s, bv_s, None)
        proj_quarter(wq_s, 0, qT_s, bq_s, SCALE)
        span_scores(0, 0)
        span_scores(0, 1)
        proj_quarter(wk_s, 1, kT_s, bk_s, SCALE)
        proj_quarter(wv_s, 1, vT_s, bv_s, None)
        span_scores(0, 2)
        span_pv(0, 0)
        proj_quarter(wk_s, 2, kT_s, bk_s, SCALE)
        proj_quarter(wv_s, 2, vT_s, bv_s, None)
        span_scores(0, 3)
        span_pv(0, 1)
        span_scores(0, 4)
        span_pv(0, 2)
        proj_quarter(wk_s, 3, kT_s, bk_s, SCALE)
        proj_quarter(wv_s, 3, vT_s, bv_s, None)
        add_q_quarter(1)
        span_scores(0, 5)
        span_pv(0, 3)
        drain_filler(3)
        span_scores(0, 6)
        span_pv(0, 4)
        drain_filler(3)
        span_scores(0, 7)
        span_pv(0, 5)
        drain_filler(len(filler))  # rest of qT quarter 1: span 1 needs it

        # Remaining spans: steady-state slots. Each slot: score pair, PV of
        # the previous pair, and a bit of filler (the NEXT q-quarter
        # projection + deferred denominator matmuls). All filler touching
        # qT quarter s+1 must be fully emitted before span s+1's scores.
        for s in range(1, NSPAN):
            span_open(s)
            if s + 1 < NSPAN:
                add_q_quarter(s + 1)
            for p in range(NPAIR):
                span_scores(s, p)
                if p == 0:
                    # finish previous span: remaining PVs + fold + lb
                    while span_state[s - 1]["pv_next"] < NPAIR:
                        span_pv(s - 1)
                    span_close(s - 1)
                else:
                    span_pv(s, p - 1)
                    drain_filler(1)
            # drain everything before the next span opens (program order!)
            drain_filler(len(filler))
        while span_state[NSPAN - 1]["pv_next"] < NPAIR:
            span_pv(NSPAN - 1)
        span_close(NSPAN - 1, defer_lb=False)
        drain_filler(len(filler))

    return nc


_CACHED = {}


def _get_nc(key="fp16"):
    if key not in _CACHED:
        nc = build_nc()
        _split_excess_waits(nc)
        _CACHED[key] = nc
    return _CACHED[key]


def _make_in_maps(x, Wq, bq, Wk, bk, Wv, bv):
    def rnd(a):
        return np.ascontiguousarray(np.asarray(a, np.float32), np.float16)

    xT = rnd(np.transpose(np.asarray(x, np.float32), (0, 2, 1)))

    def warr(w):
        w = np.asarray(w, np.float32).reshape(EC, 128, D)
        return rnd(w.transpose(1, 0, 2).reshape(128, EC * D))

    Wq, Wk, Wv = warr(Wq), warr(Wk), warr(Wv)
    bqc = np.ascontiguousarray(np.asarray(bq, np.float32))
    bkc = np.ascontiguousarray(np.asarray(bk, np.float32))
    bv = np.ascontiguousarray(np.asarray(bv, np.float32))
    ones = np.ones((128, 128), np.float16)
    return [
        {"xT": np.ascontiguousarray(xT[b]), "Wq": Wq, "Wk": Wk, "Wv": Wv,
         "bqc": bqc, "bkc": bkc, "bv": bv, "ones": ones}
        for b in range(B)
    ]


def kernel(x, Wq, bq, Wk, bk, Wv, bv, _trace=False, _mm_dt=None):
    from concourse.bass_utils import run_bass_kernel_spmd

    nc = _get_nc()
    in_maps = _make_in_maps(x, Wq, bq, Wk, bk, Wv, bv)
    res = run_bass_kernel_spmd(nc, in_maps, core_ids=list(range(B)),
                               trace=_trace)
    out = np.stack([np.ascontiguousarray(res.results[b]["outT"].T)
                    for b in range(B)])
    kernel._last_result = res
    return out


# revision 17
# speedup vs baseline: 1.0938x; 1.0663x over previous
"""Single-head attention (B=8, T=2048, E=1024, D=128) on 8 Trainium2 NeuronCores.

Strategy (data-parallel over batch, one batch element per core, all-fp16 PE):
  host: pre-transpose x -> xT[b] = x[b].T (E on rows).
  device, per core, software-pipelined so the ACT (exp) stream starts early
  and PE never idles:
    - quarter h: kT/vT projections (fp16 matmuls over 8 E-chunks, PSUM);
      quarter 0 also projects qT[0:512]; span-0 score pairs chase the kT
      quarters (flash-style over k).
    - V (natural [k, D]) from vT via DMA XBAR transposes (no PE/DVE cost).
    - per 512-wide query span: per key-block pair, two score matmuls into a
      2-bank PSUM tile; one exp [128, 1024] on ACT -> P16 fp16; PV matmuls
      (lagged one pair) accumulate OT in PSUM; DVE folds P16 into per-span
      column sums (fp16 2x); a single ones[128,128] matmul per span gives the
      softmax denominators broadcast across partitions; reciprocal + multiply
      on DVE normalize. q-quarters 1-3 projections + denominator matmuls are
      interleaved into the span slots as PE filler.
    - store outT [D, T]; host transposes back to [T, D].
"""

import os
import sys

for _p in ("/opt/trn_rl_repo",):
    if _p not in sys.path and os.path.isdir(_p):
        sys.path.append(_p)

import numpy as np

import concourse.bass as bass
import concourse.tile as tile
from concourse import mybir
from concourse.vector_clock import ScopedClock

B, T, E, D = 8, 2048, 1024, 128
EC = E // 128          # E chunks of 128 partitions
NSPAN = 4              # query spans of 512
SPAN = T // NSPAN      # 512
NKB = T // 128         # 16 key blocks
NPAIR = NKB // 2       # 8 key-block pairs
F32 = mybir.dt.float32
F32R = mybir.dt.float32r
BF16 = mybir.dt.bfloat16
F16 = mybir.dt.float16

_MAX_DRAIN_WAITS = 1


def _drain_and_barrier_split(self, tick_clock, wait_clock):
    # This walrus build rejects CTRL instructions carrying more than one sync
    # wait, so spread the kernel-tail drain's waits over single-wait NOPs.
    nc = self.nc
    collector = nc.sync.nop(nofuse=True, hint="drain_wait_collector")
    wait_clock.add_sem_waits(
        collector.ins, ScopedClock({None: tick_clock.global_clock})
    )
    si = collector.ins.sync_info
    waits = list(si.on_wait) if si and si.on_wait else []
    if len(waits) > _MAX_DRAIN_WAITS:
        si.on_wait = waits[:_MAX_DRAIN_WAITS]
        rest = waits[_MAX_DRAIN_WAITS:]
        while rest:
            chunk, rest = rest[:_MAX_DRAIN_WAITS], rest[_MAX_DRAIN_WAITS:]
            extra = nc.sync.nop(nofuse=True, hint="drain_wait_extra")
            if extra.ins.sync_info is None:
                extra.ins.sync_info = type(si)(on_wait=chunk, on_update=[])
            else:
                extra.ins.sync_info.on_wait = chunk

    nc.sync.drain()

    nc.all_engine_barrier()
    assert self.sems is not None
    popped = nc._tile_sem_poison_stack.pop()
    assert popped is self._sem_poison
    nc.clear_and_free_semaphores(list(self.sems.allocated().values()))
    nc.all_engine_barrier()


tile.TileContext._drain_and_barrier = _drain_and_barrier_split


def _split_excess_waits(nc):
    """Walrus in this env allows at most one sync wait per instruction;
    hoist extra waits onto same-engine NOPs placed just before."""
    import copy

    m = nc.m
    cnt = 0
    new_funcs = []
    for function in m.functions:
        new_function = copy.replace(function, blocks=[])
        new_function.set_allocations_from_list(function.allocations)
        for block in function.blocks:
            new_insts = []
            for inst in block.instructions:
                si = inst.sync_info
                waits = list(si.on_wait) if si and si.on_wait else []
                if len(waits) > 1:
                    for w in waits[:-1]:
                        nop = mybir.InstNoOp(name=f"I-swsplit-{cnt}",
                                             ins=[], outs=[])
                        cnt += 1
                        nop.engine = inst.engine
                        nop.sync_info = mybir.SyncInfo(on_wait=[w],
                                                       on_update=[])
                        new_insts.append(nop)
                    si.on_wait = [waits[-1]]
                new_insts.append(inst)
            new_function.blocks.append(
                copy.replace(block, instructions=new_insts))
        new_funcs.append(new_function)
    new_m = copy.replace(m, functions=[])
    for f in new_funcs:
        new_m.functions.append(f)
    nc.m = new_m
    return cnt


def build_nc():
    SCALE = float(np.float32(D) ** np.float32(-0.25))

    nc = bass.Bass()
    xT = nc.declare_dram_parameter("xT", [E, T], F16, isOutput=False)[:]
    Wq = nc.declare_dram_parameter("Wq", [128, EC * D], F16, isOutput=False)[:]
    Wk = nc.declare_dram_parameter("Wk", [128, EC * D], F16, isOutput=False)[:]
    Wv = nc.declare_dram_parameter("Wv", [128, EC * D], F16, isOutput=False)[:]
    bqc = nc.declare_dram_parameter("bqc", [D], F32, isOutput=False)[:]
    bkc = nc.declare_dram_parameter("bkc", [D], F32, isOutput=False)[:]
    bv = nc.declare_dram_parameter("bv", [D], F32, isOutput=False)[:]
    ones_d = nc.declare_dram_parameter("ones", [128, 128], F16,
                                       isOutput=False)[:]
    outT = nc.declare_dram_parameter("outT", [D, T], F32, isOutput=True)[:]

    xT_r = xT.rearrange("(c p) t -> p c t", p=128)

    with tile.TileContext(nc) as tc, \
         tc.tile_pool(name="consts", bufs=1) as consts, \
         tc.tile_pool(name="xpool", bufs=1) as xpool, \
         tc.tile_pool(name="persist", bufs=1) as persist, \
         tc.tile_pool(name="ppool", bufs=2) as ppool, \
         tc.tile_pool(name="s2pool", bufs=2) as s2pool, \
         tc.tile_pool(name="lbpool", bufs=2) as lbpool, \
         tc.tile_pool(name="outpool", bufs=2) as outpool, \
         tc.tile_pool(name="psProj", bufs=2, space="PSUM") as psProj, \
         tc.tile_pool(name="psPair", bufs=2, space="PSUM") as psPair, \
         tc.tile_pool(name="psOT", bufs=2, space="PSUM") as psOT:

        # ---- weights / constants ----
        wq_s = consts.tile([128, EC, D], F16, tag="wq")
        wk_s = consts.tile([128, EC, D], F16, tag="wk")
        wv_s = consts.tile([128, EC, D], F16, tag="wv")
        bq_s = consts.tile([128, 1], F32, tag="bq")
        bk_s = consts.tile([128, 1], F32, tag="bk")
        bv_s = consts.tile([128, 1], F32, tag="bv")
        ones_mat = consts.tile([128, 128], F16, tag="ones_mat")

        Q4 = T // 4
        xq = [xpool.tile([128, EC, Q4], F16, tag=f"xq{h}", name=f"xq{h}")
              for h in range(4)]

        # Small constants FIRST on SP (the first DVE bias-copy needs them);
        # bulk x/weight loads on GPSIMD SWDGE (0.34ns/descriptor vs ~3.8 on
        # the SP HWDGE — a [128,8,512] load is 1024 descriptors).
        for b_s, b_d in ((bk_s, bkc), (bq_s, bqc), (bv_s, bv)):
            nc.sync.dma_start(out=b_s, in_=b_d.unsqueeze(1))
        nc.sync.dma_start(out=ones_mat, in_=ones_d)
        nc.gpsimd.dma_start(out=xq[0], in_=xT_r[:, :, 0:Q4])
        nc.gpsimd.dma_start(out=wk_s,
                            in_=Wk.rearrange("p (c d) -> p c d", d=D))
        nc.gpsimd.dma_start(out=wq_s,
                            in_=Wq.rearrange("p (c d) -> p c d", d=D))
        nc.gpsimd.dma_start(out=wv_s,
                            in_=Wv.rearrange("p (c d) -> p c d", d=D))
        for h in range(1, 4):
            nc.gpsimd.dma_start(out=xq[h],
                                in_=xT_r[:, :, h * Q4:(h + 1) * Q4])

        kT_s = persist.tile([128, T], F16, tag="kT")
        vT_s = persist.tile([128, T], F16, tag="vT")
        qT_s = persist.tile([128, T], F16, tag="qT")
        V_s = persist.tile([128, NKB, D], F16, tag="V")

        def proj_quarter(w_s, h, dst, bias, scale):
            ps = psProj.tile([128, Q4], F32, tag="mm", name="proj_ps")
            hsl = slice(h * Q4, (h + 1) * Q4)
            for e in range(EC):
                nc.tensor.matmul(ps, w_s[:, e, :], xq[h][:, e, :],
                                 start=(e == 0), stop=(e == EC - 1))
            if scale is not None:
                nc.vector.tensor_scalar(out=dst[:, hsl], in0=ps, scalar1=bias,
                                  scalar2=scale, op0=mybir.AluOpType.add,
                                  op1=mybir.AluOpType.mult)
            else:
                nc.vector.tensor_scalar(out=dst[:, hsl], in0=ps, scalar1=bias,
                                        scalar2=None,
                                        op0=mybir.AluOpType.add)
            if dst is vT_s:
                # V natural layout via DMA XBAR transposes (SBUF->SBUF fp16,
                # issued on SP so the ACT sequencer never blocks on them)
                for kb in range(4 * h, 4 * h + 4):
                    nc.sync.dma_start(
                        out=V_s[:, kb, :],
                        in_=vT_s[:, kb * 128:(kb + 1) * 128],
                        transpose=True)

        # ---- PE filler machinery: deferred single-matmul emitters ----
        filler = []

        def add_q_quarter(h):
            ps_box = {}

            def mk(e):
                def emit():
                    if e == 0:
                        ps_box["ps"] = psProj.tile([128, Q4], F32, tag="mm", name="qq_ps")
                    nc.tensor.matmul(ps_box["ps"], wq_s[:, e, :],
                                     xq[h][:, e, :], start=(e == 0),
                                     stop=(e == EC - 1))
                    if e == EC - 1:
                        proj_done(h, ps_box["ps"])
                return emit

            def proj_done(h, ps):
                hsl = slice(h * Q4, (h + 1) * Q4)
                nc.vector.tensor_scalar(out=qT_s[:, hsl], in0=ps,
                                        scalar1=bq_s, scalar2=SCALE,
                                        op0=mybir.AluOpType.add,
                                        op1=mybir.AluOpType.mult)
            for e in range(EC):
                filler.append(mk(e))

        def drain_filler(n):
            for _ in range(min(n, len(filler))):
                filler.pop(0)()

        # ---- span pipeline ----
        span_state = {}

        def span_scores(s, p):
            """Emit the two score matmuls + paired exp + DVE pair-sum for
            span s, key-block pair p."""
            st = span_state[s]
            ssl = st["ssl"]
            st_ps = psPair.tile([128, 2, SPAN], F32, tag="pair", name="st_ps")
            for i in range(2):
                kb = 2 * p + i
                nc.tensor.matmul(st_ps[:, i, :],
                                 kT_s[:, kb * 128:(kb + 1) * 128],
                                 qT_s[:, ssl], start=True, stop=True)
            nc.scalar.activation(out=st["P16"][:, 2 * p:2 * p + 2, :],
                                 in_=st_ps,
                                 func=mybir.ActivationFunctionType.Exp)
            nc.vector.tensor_add(out=st["sum2"][:, p, :],
                                 in0=st["P16"][:, 2 * p, :],
                                 in1=st["P16"][:, 2 * p + 1, :])

        def span_pv(s, p=None):
            st = span_state[s]
            if p is None:
                p = st["pv_next"]
            assert p == st["pv_next"], f"PV order violation s={s} p={p}"
            st["pv_next"] += 1
            for i in range(2):
                kb = 2 * p + i
                nc.tensor.matmul(st["ot"], V_s[:, kb, :],
                                 st["P16"][:, kb, :],
                                 start=(kb == 0), stop=(kb == NKB - 1))

        def span_open(s):
            span_state[s] = {
                "ssl": slice(s * SPAN, (s + 1) * SPAN),
                "ot": psOT.tile([128, SPAN], F32, tag="ot", name="ot_ps"),
                "P16": ppool.tile([128, NKB, SPAN], F16, tag="p16", name="P16"),
                "sum2": s2pool.tile([128, NPAIR, SPAN], F16, tag="s2", name="sum2"),
                "pv_next": 0,
            }

        def span_close(s, defer_lb=True):
            """Fold pair-sums, then (deferred) denominator matmul, reciprocal,
            normalize, output DMA."""
            st = span_state[s]
            sum2 = st["sum2"]
            nc.vector.tensor_add(out=sum2[:, 0:4, :], in0=sum2[:, 0:4, :],
                                 in1=sum2[:, 4:8, :])
            nc.vector.tensor_add(out=sum2[:, 0:2, :], in0=sum2[:, 0:2, :],
                                 in1=sum2[:, 2:4, :])
            nc.vector.tensor_add(out=sum2[:, 0, :], in0=sum2[:, 0, :],
                                 in1=sum2[:, 1, :])

            def emit_lb():
                lb_ps = psProj.tile([128, SPAN], F32, tag="mm", name="lb_ps")
                nc.tensor.matmul(lb_ps, ones_mat, sum2[:, 0, :],
                                 start=True, stop=True)
                lb = lbpool.tile([128, SPAN], F32, tag="lb", name="lb")
                nc.vector.reciprocal(out=lb, in_=lb_ps)
                outsp = outpool.tile([128, SPAN], F32, tag="out", name="outsp")
                nc.vector.tensor_mul(out=outsp, in0=st["ot"], in1=lb)
                nc.sync.dma_start(out=outT[:, st["ssl"]], in_=outsp)
            if defer_lb:
                # priority: must be emitted promptly -- later spans' PV
                # accumulators WAR-wait on this span's normalize
                filler.insert(0, emit_lb)
            else:
                emit_lb()

        # ================= emission schedule =================
        # Pre-phase + flash span 0: score pairs chase the kT quarters.
        span_open(0)
        proj_quarter(wk_s, 0, kT_s, bk_s, SCALE)
        proj_quarter(wv_s, 0, vT_s, bv_s, None)
        proj_quarter(wq_s, 0, qT_s, bq_s, SCALE)
        span_scores(0, 0)
        span_scores(0, 1)
        proj_quarter(wk_s, 1, kT_s, bk_s, SCALE)
        proj_quarter(wv_s, 1, vT_s, bv_s, None)
        span_scores(0, 2)
        span_pv(0, 0)
        proj_quarter(wk_s, 2, kT_s, bk_s, SCALE)
        proj_quarter(wv_s, 2, vT_s, bv_s, None)
        span_scores(0, 3)
        span_pv(0, 1)
        span_scores(0, 4)
        span_pv(0, 2)
        proj_quarter(wk_s, 3, kT_s, bk_s, SCALE)
        proj_quarter(wv_s, 3, vT_s, bv_s, None)
        add_q_quarter(1)
        span_scores(0, 5)
        span_pv(0, 3)
        drain_filler(3)
        span_scores(0, 6)
        span_pv(0, 4)
        drain_filler(3)
        span_scores(0, 7)
        span_pv(0, 5)
        drain_filler(len(filler))  # rest of qT quarter 1: span 1 needs it

        # Remaining spans: steady-state slots. Each slot: score pair, PV of
        # the previous pair, and a bit of filler (the NEXT q-quarter
        # projection + deferred denominator matmuls). All filler touching
        # qT quarter s+1 must be fully emitted before span s+1's scores.
        for s in range(1, NSPAN):
            span_open(s)
            if s + 1 < NSPAN:
                add_q_quarter(s + 1)
            for p in range(NPAIR):
                span_scores(s, p)
                if p == 0:
                    # finish previous span: remaining PVs + fold + lb
                    while span_state[s - 1]["pv_next"] < NPAIR:
                        span_pv(s - 1)
                    span_close(s - 1)
                else:
                    span_pv(s, p - 1)
                    drain_filler(1)
            # drain everything before the next span opens (program order!)
            drain_filler(len(filler))
        while span_state[NSPAN - 1]["pv_next"] < NPAIR:
            span_pv(NSPAN - 1)
        span_close(NSPAN - 1, defer_lb=False)
        drain_filler(len(filler))

    return nc


_CACHED = {}


def _get_nc(key="fp16"):
    if key not in _CACHED:
        nc = build_nc()
        _split_excess_waits(nc)
        _CACHED[key] = nc
    return _CACHED[key]


def _make_in_maps(x, Wq, bq, Wk, bk, Wv, bv):
    def rnd(a):
        return np.ascontiguousarray(np.asarray(a, np.float32), np.float16)

    xT = rnd(np.transpose(np.asarray(x, np.float32), (0, 2, 1)))

    def warr(w):
        w = np.asarray(w, np.float32).reshape(EC, 128, D)
        return rnd(w.transpose(1, 0, 2).reshape(128, EC * D))

    Wq, Wk, Wv = warr(Wq), warr(Wk), warr(Wv)
    bqc = np.ascontiguousarray(np.asarray(bq, np.float32))
    bkc = np.ascontiguousarray(np.asarray(bk, np.float32))
    bv = np.ascontiguousarray(np.asarray(bv, np.float32))
    ones = np.ones((128, 128), np.float16)
    return [
        {"xT": np.ascontiguousarray(xT[b]), "Wq": Wq, "Wk": Wk, "Wv": Wv,
         "bqc": bqc, "bkc": bkc, "bv": bv, "ones": ones}
        for b in range(B)
    ]


def kernel(x, Wq, bq, Wk, bk, Wv, bv, _trace=False, _mm_dt=None):
    from concourse.bass_utils import run_bass_kernel_spmd

    nc = _get_nc()
    in_maps = _make_in_maps(x, Wq, bq, Wk, bk, Wv, bv)
    res = run_bass_kernel_spmd(nc, in_maps, core_ids=list(range(B)),
                               trace=_trace)
    out = np.stack([np.ascontiguousarray(res.results[b]["outT"].T)
                    for b in range(B)])
    kernel._last_result = res
    return out
